# revision 5
# baseline (speedup 1.0000x reference)
"""Multi-head graph attention (GAT) Trainium2 kernel, head-sharded across 8 NeuronCores.

Per-core program (one head per core), all in the transposed [j-part, i-free] world:
  hpT[o,n] = w.T @ h.T           (PE, fp32)
  src[i]   = a_src.T @ hpT       (PE, fp32)  -> broadcast across partitions -> SRC_b
  dst[j]   = hpT_chunk.T @ a_dst (PE, fp32)  -> dst_col [128, n/128]
  score    = (SRC_b + dst) + adjmT            (DVE scalar_tensor_tensor; adjmT = (adj.T-1)*128 in fp8)
  lrelu    = Prelu(score, alpha=0.2)          (ACT)
  e        = Exp(lrelu)                       (ACT, fp32)  -> DMA out (raw weight^T)
  er       = round(e, f32r)                   (DVE copy)
  rowsum  += ones.T @ er                      (PE f32r, PSUM accum)
  outT    += hp_chunk.T @ er                  (PE f32r, PSUM accum)  [hp = transpose(hpT) tiles]
Host: normalize by rowsum, sum heads, transpose back, add bias.
"""

import os
import sys

import numpy as np
import ml_dtypes

import concourse.bacc as bacc
import concourse.tile as tile
import concourse.mybir as mybir
from concourse.bass_utils import run_bass_kernel_spmd
from concourse.masks import make_identity

F32 = mybir.dt.float32
F32R = mybir.dt.float32r
FP8 = mybir.dt.float8e4
FP8_NP = mybir.dt.np(FP8)

P = 128
C_MASK = 128.0  # additive mask magnitude; exactly representable in fp8e4m3
ALPHA = 0.2

N_FULL, NH_FULL, F_FULL, O_FULL = 4096, 8, 1024, 128


def build_program(n=N_FULL, f=F_FULL, o=O_FULL, n_cores=8, ih=2048):
    """Build the per-core Bass program. Same NEFF on all cores (SPMD by data)."""
    assert n % P == 0 and f % P == 0 and o == P
    ih = min(ih, n)
    assert n % ih == 0 and ih % 512 == 0
    nj = n // P
    nf = f // P
    nhalves = n // ih

    nc = bacc.Bacc("TRN2", target_bir_lowering=False, debug=False,
                   num_devices=n_cores)

    hT_in = nc.dram_tensor("hT", [f, n], F32, kind="ExternalInput")
    w_in = nc.dram_tensor("w", [f, o], F32, kind="ExternalInput")
    asrc_in = nc.dram_tensor("a_src", [o, 1], F32, kind="ExternalInput")
    adst_in = nc.dram_tensor("a_dst", [o, 1], F32, kind="ExternalInput")
    adjmT_in = nc.dram_tensor("adjmT", [n, n], FP8, kind="ExternalInput")
    wT_out = nc.dram_tensor("wT_raw", [n, n], F32, kind="ExternalOutput")
    outT_out = nc.dram_tensor("outT_raw", [o, n], F32, kind="ExternalOutput")
    rs_out = nc.dram_tensor("rowsum", [1, n], F32, kind="ExternalOutput")

    with tile.TileContext(nc) as tc:
        with tc.tile_pool(name="const", bufs=1) as const:
            ident = const.tile([P, P], F32)
            make_identity(nc, ident)
            ones_f = const.tile([P, 1], F32)
            nc.vector.memset(ones_f[:], 1.0)
            onesr = const.tile([P, 1], F32R)
            nc.vector.tensor_copy(onesr[:], ones_f[:])

            # persistent SBUF tensors
            hpT = const.tile([P, n], F32)        # [o, n]
            hp_r = const.tile([P, nj * P], F32R)  # 32 x [j-part, o] tiles, f32r
            src_b = const.tile([P, n], F32)      # src broadcast across partitions
            dst_col = const.tile([P, nj], F32)   # dst[j] at [j%128, j//128]
            wts = const.tile([P, nf * o], F32)   # w tiles [f-part, o] per f-chunk
            asrc_t = const.tile([P, 1], F32)
            adst_t = const.tile([P, 1], F32)

            nc.sync.dma_start(asrc_t[:], asrc_in[:])
            nc.sync.dma_start(adst_t[:], adst_in[:])
            for fc in range(nf):
                nc.sync.dma_start(wts[:, fc * o:(fc + 1) * o],
                                  w_in[fc * P:(fc + 1) * P, :])

            # ---- Phase 1: hpT = w.T @ hT  (accumulate over f-chunks) ----
            with tc.tile_pool(name="p1psum", bufs=1, space="PSUM") as p1psum, \
                 tc.tile_pool(name="hchunk", bufs=2) as hpool:
                psum_hpT = p1psum.tile([P, n if n <= 4096 else 4096], F32)
                assert n * 4 <= 16384 + 1  # [P, n] f32 must fit in PSUM (n<=4096)
                for fc in range(nf):
                    hch = hpool.tile([P, n], F32)
                    nc.sync.dma_start(hch[:], hT_in[fc * P:(fc + 1) * P, :])
                    for b in range(n // 512):
                        sl = slice(b * 512, (b + 1) * 512)
                        nc.tensor.matmul(psum_hpT[:, sl],
                                         wts[:, fc * o:(fc + 1) * o],
                                         hch[:, sl],
                                         start=(fc == 0), stop=(fc == nf - 1))
                for b in range(n // 512):
                    sl = slice(b * 512, (b + 1) * 512)
                    nc.scalar.copy(hpT[:, sl], psum_hpT[:, sl])

            # ---- Phase 2/3: hp tiles (transpose of hpT), src/dst ----
            with tc.tile_pool(name="p2psum", bufs=2, space="PSUM") as p2psum, \
                 tc.tile_pool(name="p2sb", bufs=2) as p2sb:
                # hp_r tiles: transpose each [o, 128j] block -> [128j, o], round f32r
                for c in range(nj):
                    pt = p2psum.tile([P, P], F32, tag="tp")
                    nc.tensor.transpose(pt[:], hpT[:, c * P:(c + 1) * P], ident[:])
                    nc.vector.tensor_copy(hp_r[:, c * P:(c + 1) * P], pt[:])

                # dst_col[:, c] = hpT[:, c-block].T @ a_dst
                pd = p2psum.tile([P, nj], F32, tag="dst")
                for c in range(nj):
                    nc.tensor.matmul(pd[:, c:c + 1],
                                     hpT[:, c * P:(c + 1) * P], adst_t[:],
                                     start=True, stop=True)
                nc.vector.tensor_copy(dst_col[:], pd[:])

                # src_row[0, :] = a_src.T @ hpT ; then broadcast to 128 partitions
                ones_row = p2sb.tile([1, P], F32, tag="ones_row")
                nc.vector.memset(ones_row[:], 1.0)
                src_row = p2sb.tile([1, n], F32, tag="src_row")
                for b in range(n // 512):
                    sl = slice(b * 512, (b + 1) * 512)
                    ps = p2psum.tile([1, 512], F32, tag="srcp")
                    nc.tensor.matmul(ps[:], asrc_t[:], hpT[:, sl],
                                     start=True, stop=True)
                    nc.vector.tensor_copy(src_row[:, sl], ps[:])
                for b in range(n // 512):
                    sl = slice(b * 512, (b + 1) * 512)
                    pb = p2psum.tile([P, 512], F32, tag="bcast")
                    nc.tensor.matmul(pb[:], ones_row[:], src_row[:, sl],
                                     start=True, stop=True)
                    nc.scalar.copy(src_b[:, sl], pb[:])

            # ---- Phase 4: main streaming loop ----
            with tc.tile_pool(name="p4psum", bufs=1, space="PSUM") as p4psum, \
                 tc.tile_pool(name="adjp", bufs=3) as adjp, \
                 tc.tile_pool(name="scp", bufs=2) as scp, \
                 tc.tile_pool(name="lrp", bufs=2) as lrp, \
                 tc.tile_pool(name="ep", bufs=3) as ep, \
                 tc.tile_pool(name="erp", bufs=3) as erp, \
                 tc.tile_pool(name="outp", bufs=2) as outp:
                for h in range(nhalves):
                    isl = slice(h * ih, (h + 1) * ih)
                    psum_out = p4psum.tile([P, ih], F32, tag="po")
                    psum_rs = p4psum.tile([P, ih], F32, tag="prs")
                    for c in range(nj):
                        adjm = adjp.tile([P, ih], FP8)
                        nc.sync.dma_start(adjm[:],
                                          adjmT_in[c * P:(c + 1) * P, isl])
                        score = scp.tile([P, ih], F32)
                        nc.vector.scalar_tensor_tensor(
                            out=score[:], in0=src_b[:, isl],
                            scalar=dst_col[:, c:c + 1], in1=adjm[:],
                            op0=mybir.AluOpType.add, op1=mybir.AluOpType.add)
                        lr = lrp.tile([P, ih], F32)
                        nc.scalar.activation(lr[:], score[:],
                                             mybir.ActivationFunctionType.Prelu,
                                             bias=0.0, scale=1.0, alpha=ALPHA)
                        e = ep.tile([P, ih], F32)
                        nc.scalar.activation(e[:], lr[:],
                                             mybir.ActivationFunctionType.Exp)
                        nc.sync.dma_start(wT_out[c * P:(c + 1) * P, isl], e[:])
                        er = erp.tile([P, ih], F32R)
                        nc.vector.tensor_copy(er[:], e[:])
                        for b in range(ih // 512):
                            sl = slice(b * 512, (b + 1) * 512)
                            nc.tensor.matmul(psum_rs[0:1, sl], onesr[:],
                                             er[:, sl],
                                             start=(c == 0), stop=(c == nj - 1))
                            nc.tensor.matmul(psum_out[:, sl],
                                             hp_r[:, c * P:(c + 1) * P],
                                             er[:, sl],
                                             start=(c == 0), stop=(c == nj - 1))
                    rs_sb = outp.tile([1, ih], F32, tag="rs")
                    nc.vector.tensor_copy(rs_sb[:], psum_rs[0:1, :])
                    nc.sync.dma_start(rs_out[0:1, isl], rs_sb[:])
                    o_sb = outp.tile([P, ih], F32, tag="ot")
                    nc.scalar.copy(o_sb[:], psum_out[:])
                    nc.sync.dma_start(outT_out[:, isl], o_sb[:])

    nc.compile()
    return nc


_PROG_CACHE = {}
LAST_RESULTS = None


def _get_program(n, f, o, n_cores, ih=2048):
    key = (n, f, o, n_cores, ih)
    if key not in _PROG_CACHE:
        _PROG_CACHE[key] = build_program(n, f, o, n_cores, ih)
    return _PROG_CACHE[key]


def kernel(h, adj, w, bias, a_src, a_dst):
    n, f_in = h.shape
    n_head, _, f_out = w.shape
    n_cores = n_head
    prog = _get_program(n, f_in, f_out, n_cores)

    hT = np.ascontiguousarray(h.T).astype(np.float32, copy=False)
    adjmT = ((adj.T.astype(np.float32) - 1.0) * C_MASK).astype(FP8_NP)

    in_maps = []
    for head in range(n_head):
        in_maps.append({
            "hT": hT,
            "adjmT": adjmT,
            "w": np.ascontiguousarray(w[head]).astype(np.float32, copy=False),
            "a_src": np.ascontiguousarray(a_src[head]).astype(np.float32, copy=False),
            "a_dst": np.ascontiguousarray(a_dst[head]).astype(np.float32, copy=False),
        })

    kwargs = {}
    if os.environ.get("BASS_KERNEL_TRACE"):
        kwargs = dict(trace=True)
    res_obj = run_bass_kernel_spmd(prog, in_maps,
                                   core_ids=list(range(n_cores)), **kwargs)
    global LAST_RESULTS
    LAST_RESULTS = res_obj
    results = res_obj.results

    out = np.empty((n, n_head * f_out), np.float32)
    wacc = np.zeros((n, n), np.float32)
    tmp = np.empty((n, n), np.float32)
    bias32 = bias.astype(np.float32, copy=False)
    for head in range(n_head):
        r = results[head]
        inv = (1.0 / r["rowsum"][0]).astype(np.float32)
        np.multiply(r["wT_raw"], inv[None, :], out=tmp)
        wacc += tmp
        out[:, head * f_out:(head + 1) * f_out] = \
            (r["outT_raw"] * inv[None, :]).T + bias32[None, :]
    weight = np.ascontiguousarray(wacc.T)
    return out, weight


# revision 17
# speedup vs baseline: 233.4628x; 233.4628x over previous
"""Multi-head graph attention (GAT) Trainium2 kernel, head-sharded across 8 NeuronCores.

Per-core program (one head per core), entirely in the transposed
[j-on-partitions, i-on-free] world so no on-device transpose of the big
attention matrix is ever needed:
  hpT[o,n] = w.T @ h.T            (PE fp32, streamed per 512-col block)
  src[i]   = a_src.T @ hpT        (PE)  -> ones-matmul broadcast -> SRC_b [128, n]
  dst[j]   = hpT_block.T @ a_dst  (PE)  -> dst_col [128, n/128]
  per (i-half, j-chunk), interleaved with the block stream above:
    score = (SRC_b + dst) + adjmT      (DVE scalar_tensor_tensor;
                                        adjmT = (adj.T-1)*128 as fp8 -> exp ~ 1e-11 where adj=0)
    lrelu = max(0.2*score, score)      (alternates DVE stt / ACT Prelu(alpha=0.2) per chunk)
    e     = Exp(lrelu)                 (ACT, output rounded to float32r)
    e -> DRAM (raw weight^T partial, via SWDGE so stores don't HOL-block loads)
    outT += hp_block.T @ e             (PE float32r matmuls, 1 cyc/row, PSUM accum)
Host: rowsum = column-sums of e, normalize out/weight, sum heads, transpose, add bias.

Engine budget per core (cost model): DMA 287us (~98.5MB at ~360GB/s, the roofline),
DVE 224us, ACT 196us, PE 189us; end-to-end 304us.
"""

import os
import sys

import numpy as np
import ml_dtypes

import concourse.bacc as bacc
import concourse.tile as tile
import concourse.mybir as mybir
from concourse.bass_utils import run_bass_kernel_spmd
from concourse.masks import make_identity

F32 = mybir.dt.float32
F32R = mybir.dt.float32r
FP8 = mybir.dt.float8e4
FP8_NP = mybir.dt.np(FP8)

P = 128
C_MASK = 128.0  # additive mask magnitude; exactly representable in fp8e4m3
ALPHA = 0.2

N_FULL, NH_FULL, F_FULL, O_FULL = 4096, 8, 1024, 128


def build_program(n=N_FULL, f=F_FULL, o=O_FULL, n_cores=8, ih=None):
    """Build the per-core Bass program. Same NEFF on all cores (SPMD by data)."""
    assert n % P == 0 and f % P == 0 and o == P
    if ih is None:
        ih = int(os.environ.get("K_IH", "2048"))
    ih = min(ih, n)
    assert n % ih == 0 and ih % 512 == 0
    nj = n // P
    nf = f // P
    nhalves = n // ih

    nc = bacc.Bacc("TRN2", target_bir_lowering=False, debug=False,
                   num_devices=n_cores)

    hT_in = nc.dram_tensor("hT", [f, n], F32, kind="ExternalInput")
    w_in = nc.dram_tensor("w", [f, o], F32, kind="ExternalInput")
    asrc_in = nc.dram_tensor("a_src", [o, 1], F32, kind="ExternalInput")
    adst_in = nc.dram_tensor("a_dst", [o, 1], F32, kind="ExternalInput")
    adjmT_in = nc.dram_tensor("adjmT", [n, n], FP8, kind="ExternalInput")
    wT_out = nc.dram_tensor("wT_raw", [n, n], F32, kind="ExternalOutput")
    outT_out = nc.dram_tensor("outT_raw", [o, n], F32, kind="ExternalOutput")
    rs_out = nc.dram_tensor("rowsum", [1, n], F32, kind="ExternalOutput")

    with tile.TileContext(nc) as tc:
        with tc.tile_pool(name="const", bufs=1) as const:
            ident = const.tile([P, P], F32)
            make_identity(nc, ident)
            ones_f = const.tile([P, 1], F32)
            nc.vector.memset(ones_f[:], 1.0)
            onesr = const.tile([P, 1], F32R)
            nc.vector.tensor_copy(onesr[:], ones_f[:])

            # persistent SBUF tensors
            hpT = const.tile([P, n], F32)        # [o, n]
            hp_r = const.tile([P, nj * P], F32R)  # 32 x [j-part, o] tiles, f32r
            src_b = const.tile([P, n], F32)      # src broadcast across partitions
            dst_col = const.tile([P, nj], F32)   # dst[j] at [j%128, j//128]
            wts = const.tile([P, nf * o], F32)   # w tiles [f-part, o] per f-chunk
            asrc_t = const.tile([P, 1], F32)
            adst_t = const.tile([P, 1], F32)

            nc.sync.dma_start(asrc_t[:], asrc_in[:])
            nc.sync.dma_start(adst_t[:], adst_in[:])
            for fc in range(nf):
                nc.sync.dma_start(wts[:, fc * o:(fc + 1) * o],
                                  w_in[fc * P:(fc + 1) * P, :])

            # ---- Phases 1-3 streamed per 512-col block, interleaved with the
            # phase-4 main loop so compute starts before all of hT is processed ----
            ones_row = const.tile([1, P], F32)
            nc.vector.memset(ones_row[:], 1.0)
            src_row = const.tile([1, n], F32)
            single = ih == n or os.environ.get("K_NORS", "1") == "1"
            with tc.tile_pool(name="p4psum", bufs=int(os.environ.get("K_PSUM_BUFS", "1")), space="PSUM") as p4, \
                 tc.tile_pool(name="phpsum", bufs=2, space="PSUM") as php, \
                 tc.tile_pool(name="hchunk", bufs=int(os.environ.get("K_H_BUFS", "6"))) as hpool, \
                 tc.tile_pool(name="adjp", bufs=int(os.environ.get("K_ADJ_BUFS", "14"))) as adjp, \
                 tc.tile_pool(name="scp", bufs=int(os.environ.get("K_SC_BUFS", "3"))) as scp, \
                 tc.tile_pool(name="lrp", bufs=int(os.environ.get("K_LR_BUFS", "3"))) as lrp, \
                 tc.tile_pool(name="erp", bufs=int(os.environ.get("K_ER_BUFS", "4"))) as erp, \
                 tc.tile_pool(name="outp", bufs=2) as outp:

                def emit_block(b):
                    sl = slice(b * 512, (b + 1) * 512)
                    pb1 = php.tile([P, 512], F32, tag="hp")
                    for fc in range(nf):
                        hch = hpool.tile([P, 512], F32)
                        nc.sync.dma_start(hch[:], hT_in[fc * P:(fc + 1) * P, sl])
                        nc.tensor.matmul(pb1[:], wts[:, fc * o:(fc + 1) * o],
                                         hch[:],
                                         start=(fc == 0), stop=(fc == nf - 1))
                    nc.scalar.copy(hpT[:, sl], pb1[:])
                    # src slice + partition broadcast
                    ps = php.tile([1, 512], F32, tag="aux")
                    nc.tensor.matmul(ps[:], asrc_t[:], hpT[:, sl],
                                     start=True, stop=True)
                    nc.vector.tensor_copy(src_row[:, sl], ps[:])
                    pbc = php.tile([P, 512], F32, tag="aux")
                    nc.tensor.matmul(pbc[:], ones_row[:], src_row[:, sl],
                                     start=True, stop=True)
                    nc.scalar.copy(src_b[:, sl], pbc[:])
                    # hp transposes + dst for the four 128-col pieces
                    for q in range(4):
                        c = b * 4 + q
                        pt = php.tile([P, P], F32, tag="aux")
                        nc.tensor.transpose(pt[:], hpT[:, c * P:(c + 1) * P],
                                            ident[:])
                        nc.vector.tensor_copy(hp_r[:, c * P:(c + 1) * P], pt[:])
                        pd = php.tile([P, 1], F32, tag="aux")
                        nc.tensor.matmul(pd[:], hpT[:, c * P:(c + 1) * P],
                                         adst_t[:], start=True, stop=True)
                        nc.vector.tensor_copy(dst_col[:, c:c + 1], pd[:])

                nb = n // 512
                nb_pre = min(nb, ih // 512)
                emit_at = {4 * (i + 1): k
                           for i, k in enumerate(range(nb_pre, nb))}
                assert all(pos <= 4 * k and pos < nj
                           for pos, k in emit_at.items())
                for b in range(nb_pre):
                    emit_block(b)

                leaky_dve_parity = int(os.environ.get("K_LEAKY_SPLIT", "1"))
                store_swdge = os.environ.get("K_STORE_SWDGE", "1") == "1"
                for h in range(nhalves):
                    isl = slice(h * ih, (h + 1) * ih)
                    psum_out = p4.tile([P, ih], F32, tag="po")
                    if not single:
                        psum_rs = p4.tile([P, ih], F32, tag="prs")
                    for c in range(nj):
                        if h == 0 and c in emit_at:
                            emit_block(emit_at[c])
                        adjm = adjp.tile([P, ih], FP8)
                        nc.sync.dma_start(adjm[:],
                                          adjmT_in[c * P:(c + 1) * P, isl])
                        score = scp.tile([P, ih], F32)
                        nc.vector.scalar_tensor_tensor(
                            out=score[:], in0=src_b[:, isl],
                            scalar=dst_col[:, c:c + 1], in1=adjm[:],
                            op0=mybir.AluOpType.add, op1=mybir.AluOpType.add)
                        lr = lrp.tile([P, ih], F32)
                        if c % 2 == leaky_dve_parity:
                            # leaky on DVE: max(0.2*s, s)
                            nc.vector.scalar_tensor_tensor(
                                out=lr[:], in0=score[:], scalar=ALPHA,
                                in1=score[:], op0=mybir.AluOpType.mult,
                                op1=mybir.AluOpType.max)
                        else:
                            nc.scalar.activation(
                                lr[:], score[:],
                                mybir.ActivationFunctionType.Prelu,
                                bias=0.0, scale=1.0, alpha=ALPHA)
                        er = erp.tile([P, ih], F32R)
                        nc.scalar.activation(er[:], lr[:],
                                             mybir.ActivationFunctionType.Exp)
                        if store_swdge:
                            nc.gpsimd.dma_start(wT_out[c * P:(c + 1) * P, isl],
                                                er[:].bitcast(F32))
                        else:
                            nc.sync.dma_start(wT_out[c * P:(c + 1) * P, isl],
                                              er[:].bitcast(F32))
                        for b in range(ih // 512):
                            sl = slice(b * 512, (b + 1) * 512)
                            if not single:
                                nc.tensor.matmul(psum_rs[0:1, sl], onesr[:],
                                                 er[:, sl],
                                                 start=(c == 0), stop=(c == nj - 1))
                            nc.tensor.matmul(psum_out[:, sl],
                                             hp_r[:, c * P:(c + 1) * P],
                                             er[:, sl],
                                             start=(c == 0), stop=(c == nj - 1))
                    if not single:
                        rs_sb = outp.tile([1, ih], F32, tag="rs")
                        nc.vector.tensor_copy(rs_sb[:], psum_rs[0:1, :])
                        nc.sync.dma_start(rs_out[0:1, isl], rs_sb[:])
                    o_sb = outp.tile([P, ih], F32, tag="ot")
                    nc.scalar.copy(o_sb[:], psum_out[:])
                    nc.sync.dma_start(outT_out[:, isl], o_sb[:])

    nc.compile()
    return nc


_PROG_CACHE = {}
LAST_RESULTS = None


def _get_program(n, f, o, n_cores, ih=None):
    key = (n, f, o, n_cores, ih, os.environ.get("K_IH"), os.environ.get("K_PSUM_BUFS"),
           os.environ.get("K_ADJ_BUFS"), os.environ.get("K_ER_BUFS"))
    if key not in _PROG_CACHE:
        _PROG_CACHE[key] = build_program(n, f, o, n_cores, ih)
    return _PROG_CACHE[key]


def kernel(h, adj, w, bias, a_src, a_dst):
    h = np.asarray(h)
    adj = np.asarray(adj)
    w = np.asarray(w)
    bias = np.asarray(bias)
    a_src = np.asarray(a_src)
    a_dst = np.asarray(a_dst)
    n, f_in = h.shape
    n_head, _, f_out = w.shape
    n_cores = n_head
    prog = _get_program(n, f_in, f_out, n_cores)

    hT = np.ascontiguousarray(h.T).astype(np.float32, copy=False)
    adjmT = ((adj.T.astype(np.float32) - 1.0) * C_MASK).astype(FP8_NP)

    in_maps = []
    for head in range(n_head):
        in_maps.append({
            "hT": hT,
            "adjmT": adjmT,
            "w": np.ascontiguousarray(w[head]).astype(np.float32, copy=False),
            "a_src": np.ascontiguousarray(a_src[head]).astype(np.float32, copy=False),
            "a_dst": np.ascontiguousarray(a_dst[head]).astype(np.float32, copy=False),
        })

    kwargs = {}
    if os.environ.get("BASS_KERNEL_TRACE"):
        kwargs = dict(trace=True)
    res_obj = run_bass_kernel_spmd(prog, in_maps,
                                   core_ids=list(range(n_cores)), **kwargs)
    global LAST_RESULTS
    LAST_RESULTS = res_obj
    results = res_obj.results

    out = np.empty((n, n_head * f_out), np.float32)
    bias32 = bias.astype(np.float32, copy=False)

    from concurrent.futures import ThreadPoolExecutor

    def normalize(head):
        r = results[head]
        rs = r["rowsum"][0]
        if not rs.any():  # single-pass program: rowsum computed on host
            rs = r["wT_raw"].sum(axis=0, dtype=np.float32)
        inv = (1.0 / rs).astype(np.float32)
        wn = r["wT_raw"] * inv[None, :]
        out[:, head * f_out:(head + 1) * f_out] = \
            (r["outT_raw"] * inv[None, :]).T + bias32[None, :]
        return wn

    with ThreadPoolExecutor(max_workers=8) as pool:
        wns = list(pool.map(normalize, range(n_head)))
        # pairwise tree-sum of the 8 weight^T partials
        while len(wns) > 1:
            pairs = [(wns[i], wns[i + 1]) for i in range(0, len(wns) - 1, 2)]
            rest = [wns[-1]] if len(wns) % 2 else []
            wns = list(pool.map(lambda ab: np.add(ab[0], ab[1], out=ab[0]),
                                pairs)) + rest
    weight = wns[0].T
    return out, weight

